# revision 1
# baseline (speedup 1.0000x reference)
"""Cross-attention kernel for Trainium2, distributed over 8 NeuronCores.

Sharding: data-parallel over batch (4) x tensor-parallel over head groups (2).
Core c handles batch b = c//2, heads [4g, 4g+4) with g = c%2.

Per-core device pipeline (layouts chosen so no on-device transposes are
needed; x^T / context^T are produced host-side as part of sharding):
  qT  = tanh(Wq_g^T @ x^T) * qmask          [256, 2048]   (d on partitions)
  kT  = tanh(Wk_g^T @ ctx^T), null col, pad [256, 2176]
  v   = ctx @ Wv_g (+ null row, ones col)   [2176, 4x65]  (j on partitions)
  S^T = exp(0.125 * kT_h^T qT_h + cmbias)   per (head, jtile, ichunk)
  outT_h = v_aug^T @ S^T  (row 64 = softmax denominator)
  rank-1 correction for masked queries, divide by denominator,
  out_partial = O @ Wo_g                    [2048, 512]
Host sums the two head-group partials per batch and adds bo.

PE instructions on TRN2 can carry at most ONE sync wait (walrus S3_LW /
ENGINE_NOP structs); Tile sometimes assigns more. `_split_pe_waits` runs
after scheduling and hoists extra waits onto PE nops inserted immediately
before the offending instruction — same engine stream, same blocking
semantics.
"""

import numpy as np

import concourse.bass as bass
import concourse.tile as tile
from concourse import bacc, bass_utils, mybir

FP = mybir.dt.float32
AF = mybir.ActivationFunctionType

B, N, M, DIM = 4, 2048, 2048, 512
HEADS, DH = 8, 64
G = 2          # head groups (tensor-parallel degree)
HG = 4         # heads per group
DG = HG * DH   # 256 dims per group
JT = 17        # j tiles of 128: 2048 context + null + 127 pad
JP = JT * 128  # 2176
NEG = -50.0    # additive mask bias (exp(-50) ~ 2e-22)
SCALE = 1.0 / np.sqrt(DH)  # 0.125
IC = 4         # i chunks of 512
VW = DH + 1    # v columns per head incl. ones column (den row)

LAST_RESULTS = None
_CACHE = {}


def _build():
    nc = bacc.Bacc("TRN2", debug=False, num_devices=8, enable_partition_id=False)
    d = {}

    def inp(name, shape):
        d[name] = nc.dram_tensor(name, shape, FP, kind="ExternalInput").ap()

    inp("xT", [DIM, N])
    inp("cxT", [DIM, M])
    inp("wq", [DIM, DG])
    inp("wk", [DIM, DG])
    inp("wv", [DIM, DG])
    inp("wo", [DG, DIM])
    inp("qm", [1, N])         # query mask as f32 row
    inp("cmf", [128, JT])     # context mask, padded+null, partition-major
    inp("nk", [128, 1])       # null_key tiled x2
    inp("nv", [1, HG * DH])   # null_value tiled x4
    d["out"] = nc.dram_tensor("out", [N, DIM], FP, kind="ExternalOutput").ap()

    with tile.TileContext(nc) as tc:
        _body(tc, d)
    nc.compile()
    return nc


_SPLIT_SKIP = (
    "InstDrain", "InstUnconditionalBranch", "InstCall",
    "InstEventSemaphore", "InstRegisterMove", "InstDmaTrigger",
)


def _split_pe_waits(nc):
    """Hoist all-but-one sync waits from compute-engine instructions onto
    fresh same-engine nops placed immediately before them (TRN2 TPB
    instruction structs accept only one sync wait in walrus codegen;
    drains/branches/DMA handle waits differently)."""
    engines = {
        mybir.EngineType.PE: nc.tensor,
        mybir.EngineType.Activation: nc.scalar,
        mybir.EngineType.DVE: nc.vector,
        mybir.EngineType.Pool: nc.gpsimd,
        mybir.EngineType.SP: nc.sync,
    }
    total = 0
    for bb in nc.m.functions[0].blocks:
        new_insts = []
        for ins in bb.instructions:
            si = ins.sync_info
            eng = engines.get(getattr(ins, "engine", None))
            if (
                eng is not None
                and type(ins).__name__ not in _SPLIT_SKIP
                and si is not None
                and si.on_wait
                and len(si.on_wait) > 1
            ):
                waits = list(si.on_wait)
                for w in waits[:-1]:
                    nop = eng._isa(
                        nc.isa.Opcode.NEURON_ISA_TPB_OPCODE_ENGINE_NOP,
                        {}, None, [], [], True,
                    )
                    nop.sync_info = mybir.SyncInfo(on_wait=[w], on_update=[])
                    nc.inst_map[nop.name] = nop
                    new_insts.append(nop)
                    total += 1
                si.on_wait = waits[-1:]
            new_insts.append(ins)
        bb.instructions = new_insts
    return total


def _body(tc, d):
    nc = tc.nc

    with (
        tc.tile_pool(name="consts", bufs=1) as consts,
        tc.tile_pool(name="big", bufs=1) as big,
        tc.tile_pool(name="spool", bufs=3) as spool,
        tc.tile_pool(name="small", bufs=2) as small,
        tc.tile_pool(name="mm", bufs=2, space="PSUM") as mm_ps,
        tc.tile_pool(name="acc", bufs=4, space="PSUM") as acc_ps,
        tc.tile_pool(name="rps", bufs=1, space="PSUM") as r_ps,
    ):
        # ---- constants / inputs ----
        wq = consts.tile([128, 4, DG], FP)
        nc.sync.dma_start(wq[:], d["wq"].rearrange("(c p) d -> p c d", p=128))
        wk = consts.tile([128, 4, DG], FP)
        nc.sync.dma_start(wk[:], d["wk"].rearrange("(c p) d -> p c d", p=128))
        wv = consts.tile([128, 4, DG], FP)
        nc.sync.dma_start(wv[:], d["wv"].rearrange("(c p) d -> p c d", p=128))
        wo = consts.tile([128, 2, DIM], FP)
        nc.sync.dma_start(wo[:], d["wo"].rearrange("(c p) o -> p c o", p=128))

        xT = big.tile([128, 4, N], FP)
        nc.sync.dma_start(xT[:], d["xT"].rearrange("(c p) i -> p c i", p=128))
        cxT = big.tile([128, 4, M], FP)
        nc.sync.dma_start(cxT[:], d["cxT"].rearrange("(c p) j -> p c j", p=128))

        qmB = big.tile([128, N], FP)  # query mask broadcast to 128 partitions
        nc.sync.dma_start(qmB[:], d["qm"].to_broadcast((128, N)))
        one_minus_qm = consts.tile([1, N], FP)
        nc.sync.dma_start(one_minus_qm[:], d["qm"])
        nc.scalar.activation(one_minus_qm[:], one_minus_qm[:], AF.Identity,
                             scale=-1.0, bias=1.0)

        cmf = consts.tile([128, JT], FP)
        nc.sync.dma_start(cmf[:], d["cmf"])
        negb = consts.tile([128, 1], FP)
        nc.vector.memset(negb[:], NEG)
        cmb = consts.tile([128, JT], FP)   # 0 where attendable, NEG where masked
        nc.scalar.activation(cmb[:], cmf[:], AF.Identity, scale=-NEG, bias=negb[:])
        cmexp = consts.tile([128, JT], FP)  # exp(cmb)
        nc.scalar.activation(cmexp[:], cmb[:], AF.Exp)
        negcm = consts.tile([128, JT], FP)  # -exp(cmb)
        nc.scalar.activation(negcm[:], cmexp[:], AF.Copy, scale=-1.0)

        nk = consts.tile([128, 1], FP)
        nc.sync.dma_start(nk[:], d["nk"])

        ones_col = consts.tile([128, 1], FP)
        nc.vector.memset(ones_col[:], 1.0)
        ones_pd = consts.tile([128, DH], FP)
        nc.vector.memset(ones_pd[:], 1.0)
        inv_row = consts.tile([1, 128], FP)
        nc.vector.memset(inv_row[:], 1.0 / (M + 1))

        qT = big.tile([128, 2, N], FP)
        kT = big.tile([128, 2, JP], FP)
        vsb = big.tile([128, JT, HG, VW], FP)
        Osb = big.tile([128, 2, N], FP)

        # ---- qT projection: qT[d, i] = tanh(sum_c Wq[c, d] x[i, c]) * qm[i]
        for dc in range(2):
            for ic in range(IC):
                ps = mm_ps.tile([128, 512], FP, tag="mm", name=f"psq{dc}{ic}")
                for cc in range(4):
                    nc.tensor.matmul(
                        ps[:],
                        wq[:, cc, dc * 128:(dc + 1) * 128],
                        xT[:, cc, ic * 512:(ic + 1) * 512],
                        start=(cc == 0), stop=(cc == 3),
                    )
                dst = qT[:, dc, ic * 512:(ic + 1) * 512]
                nc.scalar.activation(dst, ps[:], AF.Tanh)
                nc.vector.tensor_mul(dst, dst, qmB[:, ic * 512:(ic + 1) * 512])

        # ---- kT projection (+ tanh), null col, zero pad
        for dc in range(2):
            for jc in range(IC):
                ps = mm_ps.tile([128, 512], FP, tag="mm", name=f"psk{dc}{jc}")
                for cc in range(4):
                    nc.tensor.matmul(
                        ps[:],
                        wk[:, cc, dc * 128:(dc + 1) * 128],
                        cxT[:, cc, jc * 512:(jc + 1) * 512],
                        start=(cc == 0), stop=(cc == 3),
                    )
                nc.scalar.activation(kT[:, dc, jc * 512:(jc + 1) * 512], ps[:], AF.Tanh)
        nc.vector.memset(kT[:, :, M + 1:JP], 0.0)
        for dc in range(2):
            nc.scalar.activation(kT[:, dc, M:M + 1], nk[:], AF.Tanh)

        # ---- v projection: v[j, d]; last col of each head block = ones (denominator)
        nc.vector.memset(vsb[:, JT - 1, :, :], 0.0)
        for jt in range(JT - 1):
            ps = mm_ps.tile([128, DG], FP, tag="mm", name=f"psv{jt}")
            for cc in range(4):
                nc.tensor.matmul(
                    ps[:],
                    cxT[:, cc, jt * 128:(jt + 1) * 128],
                    wv[:, cc, :],
                    start=(cc == 0), stop=(cc == 3),
                )
            nc.vector.tensor_copy(
                vsb[:, jt, :, 0:DH],
                ps[:].rearrange("p (h e) -> p h e", h=HG),
            )
            nc.vector.memset(vsb[:, jt, :, DH:VW], 1.0)
        # null token row (j = M) lives at partition 0 of the last j tile
        nc.sync.dma_start(vsb[0:1, JT - 1, :, 0:DH],
                          d["nv"].rearrange("a (h e) -> a h e", h=HG))
        nc.vector.memset(vsb[0:1, JT - 1, :, DH:VW], 1.0)

        # ---- correction vectors (masked queries -> uniform attention)
        # corr_h = (scb/2049) * sum_all_j v_aug  -  sum_j exp(cmb_j) v_aug_j
        # (ones column of v_aug makes the denominator slot exactly 0)
        corr = consts.tile([1, HG, VW], FP)
        ps_scb = mm_ps.tile([1, JT], FP, tag="mm")
        nc.tensor.matmul(ps_scb[:], ones_col[:], cmexp[:], start=True, stop=True)
        scbrow = consts.tile([1, JT], FP)
        scb = consts.tile([1, 1], FP)
        nc.scalar.activation(scbrow[:], ps_scb[:], AF.Copy, accum_out=scb[:])
        ps_is = mm_ps.tile([128, 1], FP, tag="mm")
        nc.tensor.matmul(ps_is[:], inv_row[:], scb[:], start=True, stop=True)
        invscb = consts.tile([128, 1], FP)
        nc.scalar.copy(invscb[:], ps_is[:])
        for h in range(HG):
            ps_c = mm_ps.tile([1, VW], FP, tag="mm", name=f"psc{h}")
            for jt in range(JT):
                nc.tensor.matmul(ps_c[:], invscb[:], vsb[:, jt, h, :],
                                 start=(jt == 0), stop=False)
            for jt in range(JT):
                nc.tensor.matmul(ps_c[:], negcm[:, jt:jt + 1], vsb[:, jt, h, :],
                                 start=False, stop=(jt == JT - 1))
            nc.scalar.copy(corr[:, h, :], ps_c[:])

        # ---- flash attention over i chunks
        for ic in range(IC):
            isl = slice(ic * 512, (ic + 1) * 512)
            po = []
            for h in range(HG):
                po.append(acc_ps.tile([128, 512], FP, tag="po", name=f"po{ic}{h}"))
            for jt in range(JT):
                for h in range(HG):
                    pss = mm_ps.tile([128, 512], FP, tag="mm", name=f"pss{ic}{jt}{h}")
                    prow = 64 * (h % 2)
                    nc.tensor.matmul(
                        pss[:],
                        kT[prow:prow + DH, h // 2, jt * 128:(jt + 1) * 128],
                        qT[prow:prow + DH, h // 2, isl],
                        start=True, stop=True,
                    )
                    Ssb = spool.tile([128, 512], FP, tag="s", name=f"s{ic}{jt}{h}")
                    nc.scalar.activation(Ssb[:], pss[:], AF.Exp,
                                         bias=cmb[:, jt:jt + 1], scale=float(SCALE))
                    nc.tensor.matmul(
                        po[h][0:VW, :],
                        vsb[:, jt, h, :],
                        Ssb[:],
                        start=(jt == 0), stop=False,
                    )
            for h in range(HG):
                # rank-1 correction for masked queries (den row gets +0)
                nc.tensor.matmul(
                    po[h][0:VW, :],
                    corr[:, h, :],
                    one_minus_qm[:, isl],
                    start=False, stop=True,
                )
                den = small.tile([128, 512], FP, tag="den")
                nc.vector.tensor_copy(den[DH:VW, :], po[h][DH:VW, :])
                nc.vector.reciprocal(den[DH:VW, :], den[DH:VW, :])
                pr = r_ps.tile([DH, 512], FP, tag="pr", name=f"pr{ic}{h}")
                nc.tensor.matmul(pr[:], ones_pd[DH:VW, 0:DH], den[DH:VW, :],
                                 start=True, stop=True)
                prs = spool.tile([DH, 512], FP, tag="prs", name=f"prs{ic}{h}")
                nc.vector.tensor_copy(prs[:], pr[:])
                if h % 2 == 0:
                    nc.vector.tensor_mul(
                        Osb[0:DH, h // 2, isl], po[h][0:DH, :], prs[:])
                else:
                    ot = small.tile([DH, 512], FP, tag="ot")
                    nc.vector.tensor_mul(ot[:], po[h][0:DH, :], prs[:])
                    nc.sync.dma_start(Osb[DH:128, h // 2, isl], ot[:])

        # ---- output projection: out[i, o] = sum_hd O[hd, i] wo[hd, o]
        for it in range(N // 128):
            pf = mm_ps.tile([128, DIM], FP, tag="mm", name=f"pf{it}")
            for dc in range(2):
                nc.tensor.matmul(
                    pf[:],
                    Osb[:, dc, it * 128:(it + 1) * 128],
                    wo[:, dc, :],
                    start=(dc == 0), stop=(dc == 1),
                )
            fo = spool.tile([128, DIM], FP, tag="fo", name=f"fo{it}")
            nc.vector.tensor_copy(fo[:], pf[:])
            nc.sync.dma_start(d["out"][it * 128:(it + 1) * 128, :], fo[:])


def _core_inputs(inputs, core):
    b, g = core // 2, core % 2
    x = np.asarray(inputs["x"], np.float32)
    context = np.asarray(inputs["context"], np.float32)
    mask = np.asarray(inputs["mask"])
    context_mask = np.asarray(inputs["context_mask"])
    Wq = np.asarray(inputs["Wq"], np.float32)
    Wkv = np.asarray(inputs["Wkv"], np.float32)
    Wo = np.asarray(inputs["Wo"], np.float32)
    null_key = np.asarray(inputs["null_key"], np.float32)
    null_value = np.asarray(inputs["null_value"], np.float32)

    gs = slice(g * DG, (g + 1) * DG)
    cm = np.zeros(JP, np.float32)
    cm[:M] = context_mask[b].astype(np.float32)
    cm[M] = 1.0
    return {
        "xT": np.ascontiguousarray(x[b].T),
        "cxT": np.ascontiguousarray(context[b].T),
        "wq": np.ascontiguousarray(Wq[:, gs]),
        "wk": np.ascontiguousarray(Wkv[:, gs]),
        "wv": np.ascontiguousarray(Wkv[:, DIM + g * DG: DIM + (g + 1) * DG]),
        "wo": np.ascontiguousarray(Wo[gs, :]),
        "qm": mask[b].astype(np.float32).reshape(1, N),
        "cmf": np.ascontiguousarray(cm.reshape(JT, 128).T),
        "nk": np.ascontiguousarray(np.tile(null_key, 2).reshape(128, 1)),
        "nv": np.ascontiguousarray(np.tile(null_value, HG).reshape(1, HG * DH)),
    }


def kernel(x, context, mask, context_mask, Wq, Wkv, Wo, bo, null_key, null_value):
    global LAST_RESULTS
    inputs = {
        "x": x, "context": context, "mask": mask, "context_mask": context_mask,
        "Wq": Wq, "Wkv": Wkv, "Wo": Wo, "bo": bo,
        "null_key": null_key, "null_value": null_value,
    }
    if "nc" not in _CACHE:
        _CACHE["nc"] = _build()
    nc = _CACHE["nc"]
    in_maps = [_core_inputs(inputs, core) for core in range(8)]
    res = bass_utils.run_bass_kernel_spmd(nc, in_maps, core_ids=list(range(8)))
    LAST_RESULTS = res
    bo_np = np.asarray(bo, np.float32)
    out = np.empty((B, N, DIM), np.float32)
    for b in range(B):
        out[b] = res.results[2 * b]["out"] + res.results[2 * b + 1]["out"] + bo_np
    return out



# revision 8
# speedup vs baseline: 3.0182x; 3.0182x over previous
"""Cross-attention kernel for Trainium2, distributed over 8 NeuronCores.

Sharding: data-parallel over batch (4) x tensor-parallel over head groups (2).
Core c handles batch b = c//2, heads [4g, 4g+4) with g = c%2.

All matmuls run in bf16 (4x the fp32 PE rate; fp32 accumulation in PSUM).
Per-core device pipeline (layouts chosen so the only transposes are 32
small PE-transposes of the attention output; x^T / context^T and all
mask/bias prep happen host-side):

  qT  = tanh(Wq_g^T @ x^T) * qmask            [256, 2048] bf16 (d on parts)
  kT  = tanh(Wk_g^T @ ctx^T), null col, pad   [256, 2176] bf16
  v   = ctx @ Wv_g (+ null row, ones col)     [2176, 4x65] bf16 (j on parts)
  S^T = exp(0.125 * kT_h^T qT_h + cmbias)     per (ic, jt, head-pair):
        scores into 2 PSUM banks, one Exp activation over both -> bf16 SBUF
  po[isub] += S_tile^T(stationary) @ v_h      [128 i, 4h x 65] PSUM
        (v as the 65-wide moving operand: 8x fewer PE cycles than moving S)
  rank-1 correction for masked queries, divide by denominator column,
  PE-transpose O -> OT [hd, i], out_partial = OT^T @ Wo_g  [2048, 512] bf16
Host sums the two head-group partials per batch (fp32) and adds bo.

PE instructions on TRN2 can carry at most ONE sync wait (walrus S3_LW /
ENGINE_NOP structs); Tile sometimes assigns more. `_split_pe_waits` runs
after scheduling and hoists extra waits onto PE nops inserted immediately
before the offending instruction — same engine stream, same blocking
semantics.
"""

import ml_dtypes
import numpy as np

import concourse.bass as bass
import concourse.tile as tile
from concourse import bacc, bass_utils, mybir

FP = mybir.dt.float32
BF = mybir.dt.bfloat16
NPBF = np.dtype(ml_dtypes.bfloat16)
AF = mybir.ActivationFunctionType

B, N, M, DIM = 4, 2048, 2048, 512
HEADS, DH = 8, 64
G = 2          # head groups (tensor-parallel degree)
HG = 4         # heads per group
DG = HG * DH   # 256 dims per group
JT = 17        # j tiles of 128: 2048 context + null + 127 pad
JP = JT * 128  # 2176
NEG = -50.0    # additive mask bias (exp(-50) ~ 2e-22)
SCALE = 1.0 / np.sqrt(DH)  # 0.125
IC = 4         # i chunks of 512
VW = DH + 1    # v columns per head incl. ones column (den row)

LAST_RESULTS = None
_CACHE = {}


def _build():
    nc = bacc.Bacc("TRN2", debug=False, num_devices=8, enable_partition_id=False)
    d = {}

    def inp(name, shape, dt=BF):
        d[name] = nc.dram_tensor(name, shape, dt, kind="ExternalInput").ap()

    inp("xT", [DIM, N])
    inp("cxT", [DIM, M])
    inp("wq", [DIM, DG])
    inp("wk", [DIM, DG])
    inp("wv", [DIM, DG])
    inp("wo", [DG, DIM])
    inp("qm", [1, N])          # query mask as bf16 row (0/1)
    inp("omq", [1, N])         # 1 - qm
    inp("cmb", [128, JT], FP)  # additive mask bias: 0 attendable, NEG masked
    inp("cmexp", [128, JT])    # exp(cmb) bf16
    inp("negcm", [128, JT])    # -exp(cmb) bf16
    inp("nkt", [128, 1])       # tanh(null_key) tiled x2
    inp("nv", [1, DG])         # null_value tiled x4
    inp("ident", [128, 128])   # identity for PE transpose
    d["out"] = nc.dram_tensor("out", [N, DIM], BF, kind="ExternalOutput").ap()

    with tile.TileContext(nc) as tc:
        _body(tc, d)
    nc.compile()
    _split_pe_waits(nc)
    return nc


_SPLIT_SKIP = (
    "InstDrain", "InstUnconditionalBranch", "InstCall",
    "InstEventSemaphore", "InstRegisterMove", "InstDmaTrigger",
)


def _split_pe_waits(nc):
    """Hoist all-but-one sync waits from compute-engine instructions onto
    fresh same-engine nops placed immediately before them (TRN2 TPB
    instruction structs accept only one sync wait in walrus codegen;
    drains/branches/DMA handle waits differently)."""
    engines = {
        mybir.EngineType.PE: nc.tensor,
        mybir.EngineType.Activation: nc.scalar,
        mybir.EngineType.DVE: nc.vector,
        mybir.EngineType.Pool: nc.gpsimd,
        mybir.EngineType.SP: nc.sync,
    }
    total = 0
    for bb in nc.m.functions[0].blocks:
        new_insts = []
        for ins in bb.instructions:
            si = ins.sync_info
            eng = engines.get(getattr(ins, "engine", None))
            if (
                eng is not None
                and type(ins).__name__ not in _SPLIT_SKIP
                and si is not None
                and si.on_wait
                and len(si.on_wait) > 1
            ):
                waits = list(si.on_wait)
                for w in waits[:-1]:
                    nop = eng._isa(
                        nc.isa.Opcode.NEURON_ISA_TPB_OPCODE_ENGINE_NOP,
                        {}, None, [], [], True,
                    )
                    nop.sync_info = mybir.SyncInfo(on_wait=[w], on_update=[])
                    nc.inst_map[nop.name] = nop
                    new_insts.append(nop)
                    total += 1
                si.on_wait = waits[-1:]
            new_insts.append(ins)
        bb.instructions = new_insts
    return total


def _body(tc, d):
    nc = tc.nc

    with (
        tc.tile_pool(name="consts", bufs=1) as consts,
        tc.tile_pool(name="big", bufs=1) as big,
        tc.tile_pool(name="spool", bufs=3) as spool,
        tc.tile_pool(name="small", bufs=4) as small,
        tc.tile_pool(name="sp", bufs=2, space="PSUM") as sp_ps,
        tc.tile_pool(name="po", bufs=4, space="PSUM") as po_ps,
    ):
        # ---- constants / inputs ----
        wq = consts.tile([128, 4, DG], BF)
        nc.sync.dma_start(wq[:], d["wq"].rearrange("(c p) d -> p c d", p=128))
        wk = consts.tile([128, 4, DG], BF)
        nc.sync.dma_start(wk[:], d["wk"].rearrange("(c p) d -> p c d", p=128))
        wv = consts.tile([128, 4, DG], BF)
        nc.sync.dma_start(wv[:], d["wv"].rearrange("(c p) d -> p c d", p=128))
        wo = consts.tile([128, 2, DIM], BF)
        nc.sync.dma_start(wo[:], d["wo"].rearrange("(c p) o -> p c o", p=128))

        xT = big.tile([128, 4, N], BF)
        nc.sync.dma_start(xT[:], d["xT"].rearrange("(c p) i -> p c i", p=128))
        cxT = big.tile([128, 4, M], BF)
        nc.sync.dma_start(cxT[:], d["cxT"].rearrange("(c p) j -> p c j", p=128))

        qmB = big.tile([128, N], BF)  # query mask broadcast to 128 partitions
        nc.sync.dma_start(qmB[:], d["qm"].to_broadcast((128, N)))
        omq = consts.tile([1, N], BF)
        nc.sync.dma_start(omq[:], d["omq"])

        cmb = consts.tile([128, JT], FP)
        nc.sync.dma_start(cmb[:], d["cmb"])
        cmexp = consts.tile([128, JT], BF)
        nc.sync.dma_start(cmexp[:], d["cmexp"])
        negcm = consts.tile([128, JT], BF)
        nc.sync.dma_start(negcm[:], d["negcm"])

        ident = consts.tile([128, 128], BF)
        nc.sync.dma_start(ident[:], d["ident"])

        ones_col = consts.tile([128, 1], BF)
        nc.vector.memset(ones_col[:], 1.0)
        inv_row = consts.tile([1, 128], FP)
        nc.vector.memset(inv_row[:], 1.0 / (M + 1))

        qT = big.tile([128, 2, N], BF)
        kT = big.tile([128, 2, JP], BF)
        vsb = big.tile([128, JT, HG, VW], BF)
        OsbT = big.tile([128, 2, N], BF)

        # ---- qT projection: qT[d, i] = tanh(sum_c Wq[c, d] x[i, c]) * qm[i]
        for dc in range(2):
            for ic in range(IC):
                ps = po_ps.tile([128, 512], FP, tag="po", name=f"psq{dc}{ic}")
                for cc in range(4):
                    nc.tensor.matmul(
                        ps[:],
                        wq[:, cc, dc * 128:(dc + 1) * 128],
                        xT[:, cc, ic * 512:(ic + 1) * 512],
                        start=(cc == 0), stop=(cc == 3),
                    )
                nc.scalar.activation(qT[:, dc, ic * 512:(ic + 1) * 512],
                                     ps[:], AF.Tanh)
            nc.vector.tensor_mul(qT[:, dc, :], qT[:, dc, :], qmB[:])

        # ---- kT projection (+ tanh), null col, zero pad
        for dc in range(2):
            for jc in range(IC):
                ps = po_ps.tile([128, 512], FP, tag="po", name=f"psk{dc}{jc}")
                for cc in range(4):
                    nc.tensor.matmul(
                        ps[:],
                        wk[:, cc, dc * 128:(dc + 1) * 128],
                        cxT[:, cc, jc * 512:(jc + 1) * 512],
                        start=(cc == 0), stop=(cc == 3),
                    )
                nc.scalar.activation(kT[:, dc, jc * 512:(jc + 1) * 512],
                                     ps[:], AF.Tanh)
        nc.vector.memset(kT[:, :, M + 1:JP], 0.0)
        for dc in range(2):
            nc.sync.dma_start(kT[:, dc, M:M + 1], d["nkt"])

        # ---- v projection: v[j, d]; col 64 of each head block = ones (den)
        nc.vector.memset(vsb[:, JT - 1, :, :], 0.0)
        for jt in range(JT - 1):
            ps = po_ps.tile([128, DG], FP, tag="po", name=f"psv{jt}")
            for cc in range(4):
                nc.tensor.matmul(
                    ps[:],
                    cxT[:, cc, jt * 128:(jt + 1) * 128],
                    wv[:, cc, :],
                    start=(cc == 0), stop=(cc == 3),
                )
            nc.vector.tensor_copy(
                vsb[:, jt, :, 0:DH],
                ps[:].rearrange("p (h e) -> p h e", h=HG),
            )
            nc.vector.memset(vsb[:, jt, :, DH:VW], 1.0)
        # null token row (j = M) lives at partition 0 of the last j tile
        nc.sync.dma_start(vsb[0:1, JT - 1, :, 0:DH],
                          d["nv"].rearrange("a (h e) -> a h e", h=HG))
        nc.vector.memset(vsb[0:1, JT - 1, :, DH:VW], 1.0)

        # ---- correction vector (masked queries -> uniform attention)
        # corr = (scb/2049) * sum_all_j v_aug  -  sum_j exp(cmb_j) v_aug_j
        # (ones column of v_aug makes the denominator slot ~scb - scb = 0,
        #  which corrects the masked-query denominator to exactly scb)
        ps_scb = po_ps.tile([1, JT], FP, tag="po", name="ps_scb")
        nc.tensor.matmul(ps_scb[:], ones_col[:], cmexp[:], start=True, stop=True)
        scbrow = small.tile([1, JT], FP, tag="scb")
        scb = consts.tile([1, 1], FP)
        nc.scalar.activation(scbrow[:], ps_scb[:], AF.Copy, accum_out=scb[:])
        ps_is = po_ps.tile([128, 1], FP, tag="po", name="ps_is")
        nc.tensor.matmul(ps_is[:], inv_row[:], scb[:], start=True, stop=True)
        invscb = consts.tile([128, 1], BF)
        nc.scalar.copy(invscb[:], ps_is[:])
        ps_c = po_ps.tile([1, HG * VW], FP, tag="po", name="ps_corr")
        for jt in range(JT):
            nc.tensor.matmul(ps_c[:], invscb[:],
                             vsb[:, jt, :, :].rearrange("p h e -> p (h e)"),
                             start=(jt == 0), stop=False)
        for jt in range(JT):
            nc.tensor.matmul(ps_c[:], negcm[:, jt:jt + 1],
                             vsb[:, jt, :, :].rearrange("p h e -> p (h e)"),
                             start=False, stop=(jt == JT - 1))
        corr = consts.tile([1, HG * VW], BF)
        nc.scalar.copy(corr[:], ps_c[:])

        # ---- flash attention over i chunks of 512
        for ic in range(IC):
            isl = slice(ic * 512, (ic + 1) * 512)
            po = []
            for isub in range(4):
                po.append(po_ps.tile([128, HG, VW], FP, tag="po",
                                     name=f"po{ic}{isub}"))
            for jt in range(JT):
                for hp in range(2):
                    sps = sp_ps.tile([128, 2, 512], FP, tag="sp",
                                     name=f"sp{ic}{jt}{hp}")
                    for hh in range(2):
                        h = 2 * hp + hh
                        prow = DH * (h % 2)
                        nc.tensor.matmul(
                            sps[:, hh, :],
                            kT[prow:prow + DH, h // 2, jt * 128:(jt + 1) * 128],
                            qT[prow:prow + DH, h // 2, isl],
                            start=True, stop=True,
                        )
                    Ssb = spool.tile([128, 2, 512], BF, tag="s",
                                     name=f"s{ic}{jt}{hp}")
                    nc.scalar.activation(Ssb[:], sps[:], AF.Exp,
                                         bias=cmb[:, jt:jt + 1],
                                         scale=float(SCALE))
                    for isub in range(4):
                        for hh in range(2):
                            h = 2 * hp + hh
                            # start=True zeroes the whole 2KB PSUM zero
                            # region, so only the first matmul into each
                            # po bank may carry it.
                            nc.tensor.matmul(
                                po[isub][:, h, :],
                                Ssb[:, hh, isub * 128:(isub + 1) * 128],
                                vsb[:, jt, h, :],
                                start=(jt == 0 and h == 0), stop=False,
                            )
            # rank-1 correction for masked queries + finish accumulation
            for isub in range(4):
                i0 = ic * 512 + isub * 128
                nc.tensor.matmul(
                    po[isub][:].rearrange("p h e -> p (h e)"),
                    omq[:, i0:i0 + 128],
                    corr[:],
                    start=False, stop=True,
                )
            # divide by denominator (col 64 of each head block)
            Ods = []
            for isub in range(4):
                den = small.tile([128, HG], FP, tag="den", name=f"dn{ic}{isub}")
                nc.vector.tensor_copy(den[:], po[isub][:, :, DH])
                rden = small.tile([128, HG], FP, tag="rdn", name=f"rd{ic}{isub}")
                nc.vector.reciprocal(rden[:], den[:])
                Od = small.tile([128, HG, DH], BF, tag="od", name=f"od{ic}{isub}")
                for h in range(HG):
                    nc.vector.tensor_scalar_mul(
                        Od[:, h, :], po[isub][:, h, 0:DH], rden[:, h:h + 1])
                Ods.append(Od)
            # transpose O [i, hd] -> OT [hd, i] via PE, then copy to SBUF
            # (each transpose needs its own PSUM zero region: start=True
            #  zeroes the whole 2KB region)
            for isub in range(4):
                i0 = ic * 512 + isub * 128
                for hp in range(2):
                    pT = po_ps.tile([128, 128], BF, tag="po",
                                    name=f"pT{ic}{isub}{hp}")
                    nc.tensor.transpose(
                        pT[:],
                        Ods[isub][:, 2 * hp:2 * hp + 2, :]
                        .rearrange("p h e -> p (h e)"),
                        ident[:],
                    )
                    nc.vector.tensor_copy(OsbT[:, hp, i0:i0 + 128], pT[:])
            # output projection for this chunk
            for isub in range(4):
                i0 = ic * 512 + isub * 128
                pf = po_ps.tile([128, DIM], FP, tag="po", name=f"pf{ic}{isub}")
                for dc in range(2):
                    nc.tensor.matmul(
                        pf[:],
                        OsbT[:, dc, i0:i0 + 128],
                        wo[:, dc, :],
                        start=(dc == 0), stop=(dc == 1),
                    )
                fo = spool.tile([128, DIM], BF, tag="fo", name=f"fo{ic}{isub}")
                nc.vector.tensor_copy(fo[:], pf[:])
                nc.sync.dma_start(d["out"][i0:i0 + 128, :], fo[:])


def _core_inputs(inputs, core):
    b, g = core // 2, core % 2
    x = np.asarray(inputs["x"], np.float32)
    context = np.asarray(inputs["context"], np.float32)
    mask = np.asarray(inputs["mask"])
    context_mask = np.asarray(inputs["context_mask"])
    Wq = np.asarray(inputs["Wq"], np.float32)
    Wkv = np.asarray(inputs["Wkv"], np.float32)
    Wo = np.asarray(inputs["Wo"], np.float32)
    null_key = np.asarray(inputs["null_key"], np.float32)
    null_value = np.asarray(inputs["null_value"], np.float32)

    gs = slice(g * DG, (g + 1) * DG)
    qmf = mask[b].astype(np.float32)
    cmbf = np.full(JP, NEG, np.float32)
    cmbf[:M] = np.where(context_mask[b], 0.0, NEG).astype(np.float32)
    cmbf[M] = 0.0
    cmb = np.ascontiguousarray(cmbf.reshape(JT, 128).T)
    cmexp = np.exp(cmb)
    return {
        "xT": np.ascontiguousarray(x[b].T).astype(NPBF),
        "cxT": np.ascontiguousarray(context[b].T).astype(NPBF),
        "wq": np.ascontiguousarray(Wq[:, gs]).astype(NPBF),
        "wk": np.ascontiguousarray(Wkv[:, gs]).astype(NPBF),
        "wv": np.ascontiguousarray(
            Wkv[:, DIM + g * DG: DIM + (g + 1) * DG]).astype(NPBF),
        "wo": np.ascontiguousarray(Wo[gs, :]).astype(NPBF),
        "qm": qmf.reshape(1, N).astype(NPBF),
        "omq": (1.0 - qmf).reshape(1, N).astype(NPBF),
        "cmb": cmb,
        "cmexp": cmexp.astype(NPBF),
        "negcm": (-cmexp).astype(NPBF),
        "nkt": np.tanh(np.tile(null_key, 2)).reshape(128, 1).astype(NPBF),
        "nv": np.tile(null_value, HG).reshape(1, DG).astype(NPBF),
        "ident": np.eye(128, dtype=np.float32).astype(NPBF),
    }


def kernel(x, context, mask, context_mask, Wq, Wkv, Wo, bo, null_key, null_value):
    global LAST_RESULTS
    inputs = {
        "x": x, "context": context, "mask": mask, "context_mask": context_mask,
        "Wq": Wq, "Wkv": Wkv, "Wo": Wo, "bo": bo,
        "null_key": null_key, "null_value": null_value,
    }
    if "nc" not in _CACHE:
        _CACHE["nc"] = _build()
    nc = _CACHE["nc"]
    in_maps = [_core_inputs(inputs, core) for core in range(8)]
    res = bass_utils.run_bass_kernel_spmd(nc, in_maps, core_ids=list(range(8)))
    LAST_RESULTS = res
    bo_np = np.asarray(bo, np.float32)
    out = np.empty((B, N, DIM), np.float32)
    for b in range(B):
        out[b] = (res.results[2 * b]["out"].astype(np.float32)
                  + res.results[2 * b + 1]["out"].astype(np.float32) + bo_np)
    return out


# revision 11
# speedup vs baseline: 3.1586x; 1.0465x over previous
"""Cross-attention kernel for Trainium2, distributed over 8 NeuronCores.

Sharding: data-parallel over batch (4) x tensor-parallel over head groups (2).
Core c handles batch b = c//2, heads [4g, 4g+4) with g = c%2.

All matmuls run in bf16 (4x the fp32 PE rate; fp32 accumulation in PSUM).
Per-core device pipeline (layouts chosen so the only transposes are 32
small PE-transposes of the attention output; x^T / context^T and all
mask/bias prep happen host-side):

  qT  = tanh(Wq_g^T @ x^T) * qmask            [256, 2048] bf16 (d on parts)
  kT  = tanh(Wk_g^T @ ctx^T), null col, pad   [256, 2176] bf16
  v   = ctx @ Wv_g (+ null row, ones col)     [2176, 4x65] bf16 (j on parts)
  S^T = exp(0.125 * kT_h^T qT_h + cmbias)     per (ic, jt, head-pair):
        scores into 2 PSUM banks, one Exp activation over both -> bf16 SBUF
  po[isub] += S_tile^T(stationary) @ v_h      [128 i, 4h x 65] PSUM
        (v as the 65-wide moving operand: 8x fewer PE cycles than moving S)
  rank-1 correction for masked queries, divide by denominator column,
  PE-transpose O -> OT [hd, i], out_partial = OT^T @ Wo_g  [2048, 512] bf16
Host sums the two head-group partials per batch (fp32) and adds bo.

PE instructions on TRN2 can carry at most ONE sync wait (walrus S3_LW /
ENGINE_NOP structs); Tile sometimes assigns more. `_split_pe_waits` runs
after scheduling and hoists extra waits onto PE nops inserted immediately
before the offending instruction — same engine stream, same blocking
semantics.
"""

import ml_dtypes
import numpy as np

import concourse.bass as bass
import concourse.tile as tile
from concourse import bacc, bass_utils, mybir

FP = mybir.dt.float32
BF = mybir.dt.bfloat16
NPBF = np.dtype(ml_dtypes.bfloat16)
AF = mybir.ActivationFunctionType

B, N, M, DIM = 4, 2048, 2048, 512
HEADS, DH = 8, 64
G = 2          # head groups (tensor-parallel degree)
HG = 4         # heads per group
DG = HG * DH   # 256 dims per group
JT = 17        # j tiles of 128: 2048 context + null + 127 pad
JP = JT * 128  # 2176
NEG = -50.0    # additive mask bias (exp(-50) ~ 2e-22)
SCALE = 1.0 / np.sqrt(DH)  # 0.125
IC = 4         # i chunks of 512
VW = DH + 1    # v columns per head incl. ones column (den row)

LAST_RESULTS = None
_CACHE = {}


def _build():
    nc = bacc.Bacc("TRN2", debug=False, num_devices=8, enable_partition_id=False)
    d = {}

    def inp(name, shape, dt=BF):
        d[name] = nc.dram_tensor(name, shape, dt, kind="ExternalInput").ap()

    inp("xT", [DIM, N])
    inp("cxT", [DIM, M])
    inp("wq", [DIM, DG])
    inp("wk", [DIM, DG])
    inp("wv", [DIM, DG])
    inp("wo", [DG, DIM])
    inp("qm", [1, N])          # query mask as bf16 row (0/1)
    inp("omq", [1, N])         # 1 - qm
    inp("cmb", [128, JT], FP)  # additive mask bias: 0 attendable, NEG masked
    inp("cmexp", [128, JT])    # exp(cmb) bf16
    inp("negcm", [128, JT])    # -exp(cmb) bf16
    inp("nkt", [128, 1])       # tanh(null_key) tiled x2
    inp("nv", [1, DG])         # null_value tiled x4
    inp("ident", [128, 128])   # identity for PE transpose
    d["out"] = nc.dram_tensor("out", [N, DIM], BF, kind="ExternalOutput").ap()

    with tile.TileContext(nc) as tc:
        _body(tc, d)
    nc.compile()
    _split_pe_waits(nc)
    return nc


_SPLIT_SKIP = (
    "InstDrain", "InstUnconditionalBranch", "InstCall",
    "InstEventSemaphore", "InstRegisterMove", "InstDmaTrigger",
)


def _split_pe_waits(nc):
    """Hoist all-but-one sync waits from compute-engine instructions onto
    fresh same-engine nops placed immediately before them (TRN2 TPB
    instruction structs accept only one sync wait in walrus codegen;
    drains/branches/DMA handle waits differently)."""
    engines = {
        mybir.EngineType.PE: nc.tensor,
        mybir.EngineType.Activation: nc.scalar,
        mybir.EngineType.DVE: nc.vector,
        mybir.EngineType.Pool: nc.gpsimd,
        mybir.EngineType.SP: nc.sync,
    }
    total = 0
    for bb in nc.m.functions[0].blocks:
        new_insts = []
        for ins in bb.instructions:
            si = ins.sync_info
            eng = engines.get(getattr(ins, "engine", None))
            if (
                eng is not None
                and type(ins).__name__ not in _SPLIT_SKIP
                and si is not None
                and si.on_wait
                and len(si.on_wait) > 1
            ):
                waits = list(si.on_wait)
                for w in waits[:-1]:
                    nop = eng._isa(
                        nc.isa.Opcode.NEURON_ISA_TPB_OPCODE_ENGINE_NOP,
                        {}, None, [], [], True,
                    )
                    nop.sync_info = mybir.SyncInfo(on_wait=[w], on_update=[])
                    nc.inst_map[nop.name] = nop
                    new_insts.append(nop)
                    total += 1
                si.on_wait = waits[-1:]
            new_insts.append(ins)
        bb.instructions = new_insts
    return total


def _body(tc, d):
    nc = tc.nc

    with (
        tc.tile_pool(name="consts", bufs=1) as consts,
        tc.tile_pool(name="big", bufs=1) as big,
        tc.tile_pool(name="spool", bufs=3) as spool,
        tc.tile_pool(name="small", bufs=4) as small,
        tc.tile_pool(name="sp", bufs=2, space="PSUM") as sp_ps,
        tc.tile_pool(name="po", bufs=4, space="PSUM") as po_ps,
    ):
        # ---- constants / inputs ----
        # DMA order = first-use order: q-proj needs wq + xT chunk 0 first;
        # xT/cxT are split into 512-column chunks so the first projection
        # matmuls start while later chunks are still in flight.
        wq = consts.tile([128, 4, DG], BF)
        nc.sync.dma_start(wq[:], d["wq"].rearrange("(c p) d -> p c d", p=128))

        xT = big.tile([128, 4, N], BF)
        xTd = d["xT"].rearrange("(c p) i -> p c i", p=128)
        cxT = big.tile([128, 4, M], BF)
        cxTd = d["cxT"].rearrange("(c p) j -> p c j", p=128)
        nc.sync.dma_start(xT[:, :, 0:512], xTd[:, :, 0:512])

        wk = consts.tile([128, 4, DG], BF)
        nc.sync.dma_start(wk[:], d["wk"].rearrange("(c p) d -> p c d", p=128))
        qmB = big.tile([128, N], BF)  # query mask broadcast to 128 partitions
        nc.sync.dma_start(qmB[:], d["qm"].to_broadcast((128, N)))

        for ch in range(IC):
            csl = slice(ch * 512, (ch + 1) * 512)
            if ch > 0:
                nc.sync.dma_start(xT[:, :, csl], xTd[:, :, csl])
            nc.sync.dma_start(cxT[:, :, csl], cxTd[:, :, csl])

        wv = consts.tile([128, 4, DG], BF)
        nc.sync.dma_start(wv[:], d["wv"].rearrange("(c p) d -> p c d", p=128))
        wo = consts.tile([128, 2, DIM], BF)
        nc.sync.dma_start(wo[:], d["wo"].rearrange("(c p) o -> p c o", p=128))

        omq = consts.tile([1, N], BF)
        nc.sync.dma_start(omq[:], d["omq"])

        cmb = consts.tile([128, JT], FP)
        nc.sync.dma_start(cmb[:], d["cmb"])
        cmexp = consts.tile([128, JT], BF)
        nc.sync.dma_start(cmexp[:], d["cmexp"])
        negcm = consts.tile([128, JT], BF)
        nc.sync.dma_start(negcm[:], d["negcm"])

        ident = consts.tile([128, 128], BF)
        nc.sync.dma_start(ident[:], d["ident"])

        ones_col = consts.tile([128, 1], BF)
        nc.vector.memset(ones_col[:], 1.0)
        inv_row = consts.tile([1, 128], FP)
        nc.vector.memset(inv_row[:], 1.0 / (M + 1))

        qT = big.tile([128, 2, N], BF)
        kT = big.tile([128, 2, JP], BF)
        vsb = big.tile([128, JT, HG, VW], BF)
        OsbT = big.tile([128, 2, N], BF)

        # ---- qT projection: qT[d, i] = tanh(sum_c Wq[c, d] x[i, c]) * qm[i]
        for dc in range(2):
            for ic in range(IC):
                ps = po_ps.tile([128, 512], FP, tag="po", name=f"psq{dc}{ic}")
                for cc in range(4):
                    nc.tensor.matmul(
                        ps[:],
                        wq[:, cc, dc * 128:(dc + 1) * 128],
                        xT[:, cc, ic * 512:(ic + 1) * 512],
                        start=(cc == 0), stop=(cc == 3),
                    )
                nc.scalar.activation(qT[:, dc, ic * 512:(ic + 1) * 512],
                                     ps[:], AF.Tanh)
            nc.vector.tensor_mul(qT[:, dc, :], qT[:, dc, :], qmB[:])

        # ---- kT projection (+ tanh), null col, zero pad
        for dc in range(2):
            for jc in range(IC):
                ps = po_ps.tile([128, 512], FP, tag="po", name=f"psk{dc}{jc}")
                for cc in range(4):
                    nc.tensor.matmul(
                        ps[:],
                        wk[:, cc, dc * 128:(dc + 1) * 128],
                        cxT[:, cc, jc * 512:(jc + 1) * 512],
                        start=(cc == 0), stop=(cc == 3),
                    )
                nc.scalar.activation(kT[:, dc, jc * 512:(jc + 1) * 512],
                                     ps[:], AF.Tanh)
        nc.vector.memset(kT[:, :, M + 1:JP], 0.0)
        for dc in range(2):
            nc.sync.dma_start(kT[:, dc, M:M + 1], d["nkt"])

        # ---- v projection: v[j, d]; col 64 of each head block = ones (den)
        nc.vector.memset(vsb[:, JT - 1, :, :], 0.0)
        for jt in range(JT - 1):
            ps = po_ps.tile([128, DG], FP, tag="po", name=f"psv{jt}")
            for cc in range(4):
                nc.tensor.matmul(
                    ps[:],
                    cxT[:, cc, jt * 128:(jt + 1) * 128],
                    wv[:, cc, :],
                    start=(cc == 0), stop=(cc == 3),
                )
            nc.vector.tensor_copy(
                vsb[:, jt, :, 0:DH],
                ps[:].rearrange("p (h e) -> p h e", h=HG),
            )
            nc.vector.memset(vsb[:, jt, :, DH:VW], 1.0)
        # null token row (j = M) lives at partition 0 of the last j tile
        nc.sync.dma_start(vsb[0:1, JT - 1, :, 0:DH],
                          d["nv"].rearrange("a (h e) -> a h e", h=HG))
        nc.vector.memset(vsb[0:1, JT - 1, :, DH:VW], 1.0)

        # ---- correction vector (masked queries -> uniform attention)
        # corr = (scb/2049) * sum_all_j v_aug  -  sum_j exp(cmb_j) v_aug_j
        # (ones column of v_aug makes the denominator slot ~scb - scb = 0,
        #  which corrects the masked-query denominator to exactly scb)
        ps_scb = po_ps.tile([1, JT], FP, tag="po", name="ps_scb")
        nc.tensor.matmul(ps_scb[:], ones_col[:], cmexp[:], start=True, stop=True)
        scbrow = small.tile([1, JT], FP, tag="scb")
        scb = consts.tile([1, 1], FP)
        nc.scalar.activation(scbrow[:], ps_scb[:], AF.Copy, accum_out=scb[:])
        ps_is = po_ps.tile([128, 1], FP, tag="po", name="ps_is")
        nc.tensor.matmul(ps_is[:], inv_row[:], scb[:], start=True, stop=True)
        invscb = consts.tile([128, 1], BF)
        nc.scalar.copy(invscb[:], ps_is[:])
        ps_c = po_ps.tile([1, HG * VW], FP, tag="po", name="ps_corr")
        for jt in range(JT):
            nc.tensor.matmul(ps_c[:], invscb[:],
                             vsb[:, jt, :, :].rearrange("p h e -> p (h e)"),
                             start=(jt == 0), stop=False)
        for jt in range(JT):
            nc.tensor.matmul(ps_c[:], negcm[:, jt:jt + 1],
                             vsb[:, jt, :, :].rearrange("p h e -> p (h e)"),
                             start=False, stop=(jt == JT - 1))
        corr = consts.tile([1, HG * VW], BF)
        nc.scalar.copy(corr[:], ps_c[:])

        # ---- flash attention over i chunks of 512 ----
        # The previous chunk's epilogue (divide / transpose / out-proj) is
        # software-pipelined into this chunk's score/exp/AV stream so the
        # Exp activations (the bottleneck engine) never stall. Epilogue
        # PSUM (pT/pf) lives in the sp pool: its consumers never depend on
        # future work, so sharing the score ring cannot deadlock, while the
        # po ring stays reserved for the live accumulators.
        def emit_div(ic, po):
            Ods = []
            for isub in range(4):
                den = small.tile([128, HG], FP, tag="den", name=f"dn{ic}{isub}")
                nc.vector.tensor_copy(den[:], po[isub][:, :, DH])
                rden = small.tile([128, HG], FP, tag="rdn", name=f"rd{ic}{isub}")
                nc.vector.reciprocal(rden[:], den[:])
                Od = small.tile([128, HG, DH], BF, tag="od", name=f"od{ic}{isub}")
                for h in range(HG):
                    nc.vector.tensor_scalar_mul(
                        Od[:, h, :], po[isub][:, h, 0:DH], rden[:, h:h + 1])
                Ods.append(Od)
            return Ods

        def emit_transpose(ic, Ods, last):
            # O [i, hd] -> OT [hd, i]; both transposes share one PSUM zero
            # region via an explicit start/stop accumulation group
            for isub in range(4):
                i0 = ic * 512 + isub * 128
                pT = sp_ps.tile([128, 2, 128], BF, tag="sp",
                                name=f"pT{ic}{isub}")
                for hp in range(2):
                    nc.tensor.matmul(
                        pT[:, hp, :],
                        Ods[isub][:, 2 * hp:2 * hp + 2, :]
                        .rearrange("p h e -> p (h e)"),
                        ident[:], is_transpose=True,
                        start=(hp == 0), stop=(hp == 1),
                    )
                if last and isub % 2 == 0:
                    nc.scalar.copy(OsbT[:, :, i0:i0 + 128], pT[:])
                else:
                    nc.vector.tensor_copy(OsbT[:, :, i0:i0 + 128], pT[:])

        def emit_outproj(ic, last):
            for isub in range(4):
                i0 = ic * 512 + isub * 128
                pf = sp_ps.tile([128, DIM], FP, tag="sp", name=f"pf{ic}{isub}")
                for dc in range(2):
                    nc.tensor.matmul(
                        pf[:],
                        OsbT[:, dc, i0:i0 + 128],
                        wo[:, dc, :],
                        start=(dc == 0), stop=(dc == 1),
                    )
                fo = spool.tile([128, DIM], BF, tag="fo", name=f"fo{ic}{isub}")
                if last and isub % 2 == 0:
                    nc.scalar.copy(fo[:], pf[:])
                else:
                    nc.vector.tensor_copy(fo[:], pf[:])
                nc.sync.dma_start(d["out"][i0:i0 + 128, :], fo[:])

        prev = None  # (ic, po tiles) pending epilogue
        for ic in range(IC):
            isl = slice(ic * 512, (ic + 1) * 512)
            pOds = emit_div(prev[0], prev[1]) if prev is not None else None
            po = []
            for isub in range(4):
                po.append(po_ps.tile([128, HG, VW], FP, tag="po",
                                     name=f"po{ic}{isub}"))
            for jt in range(JT):
                for hp in range(2):
                    sps = sp_ps.tile([128, 2, 512], FP, tag="sp",
                                     name=f"sp{ic}{jt}{hp}")
                    for hh in range(2):
                        h = 2 * hp + hh
                        prow = DH * (h % 2)
                        nc.tensor.matmul(
                            sps[:, hh, :],
                            kT[prow:prow + DH, h // 2, jt * 128:(jt + 1) * 128],
                            qT[prow:prow + DH, h // 2, isl],
                            start=True, stop=True,
                        )
                    Ssb = spool.tile([128, 2, 512], BF, tag="s",
                                     name=f"s{ic}{jt}{hp}")
                    nc.scalar.activation(Ssb[:], sps[:], AF.Exp,
                                         bias=cmb[:, jt:jt + 1],
                                         scale=float(SCALE))
                    for isub in range(4):
                        for hh in range(2):
                            h = 2 * hp + hh
                            # start=True zeroes the whole 2KB PSUM zero
                            # region, so only the first matmul into each
                            # po bank may carry it.
                            nc.tensor.matmul(
                                po[isub][:, h, :],
                                Ssb[:, hh, isub * 128:(isub + 1) * 128],
                                vsb[:, jt, h, :],
                                start=(jt == 0 and h == 0), stop=False,
                            )
                if jt == 0 and pOds is not None:
                    emit_transpose(prev[0], pOds, last=False)
                elif jt == 1 and pOds is not None:
                    emit_outproj(prev[0], last=False)
            # rank-1 correction for masked queries + finish accumulation
            for isub in range(4):
                i0 = ic * 512 + isub * 128
                nc.tensor.matmul(
                    po[isub][:].rearrange("p h e -> p (h e)"),
                    omq[:, i0:i0 + 128],
                    corr[:],
                    start=False, stop=True,
                )
            prev = (ic, po)
        # final epilogue (nothing left to overlap with: split the copies
        # between ACT and DVE)
        Ods = emit_div(prev[0], prev[1])
        emit_transpose(prev[0], Ods, last=True)
        emit_outproj(prev[0], last=True)


def _core_inputs(inputs, core):
    b, g = core // 2, core % 2
    x = np.asarray(inputs["x"], np.float32)
    context = np.asarray(inputs["context"], np.float32)
    mask = np.asarray(inputs["mask"])
    context_mask = np.asarray(inputs["context_mask"])
    Wq = np.asarray(inputs["Wq"], np.float32)
    Wkv = np.asarray(inputs["Wkv"], np.float32)
    Wo = np.asarray(inputs["Wo"], np.float32)
    null_key = np.asarray(inputs["null_key"], np.float32)
    null_value = np.asarray(inputs["null_value"], np.float32)

    gs = slice(g * DG, (g + 1) * DG)
    qmf = mask[b].astype(np.float32)
    cmbf = np.full(JP, NEG, np.float32)
    cmbf[:M] = np.where(context_mask[b], 0.0, NEG).astype(np.float32)
    cmbf[M] = 0.0
    cmb = np.ascontiguousarray(cmbf.reshape(JT, 128).T)
    cmexp = np.exp(cmb)
    return {
        "xT": np.ascontiguousarray(x[b].T).astype(NPBF),
        "cxT": np.ascontiguousarray(context[b].T).astype(NPBF),
        "wq": np.ascontiguousarray(Wq[:, gs]).astype(NPBF),
        "wk": np.ascontiguousarray(Wkv[:, gs]).astype(NPBF),
        "wv": np.ascontiguousarray(
            Wkv[:, DIM + g * DG: DIM + (g + 1) * DG]).astype(NPBF),
        "wo": np.ascontiguousarray(Wo[gs, :]).astype(NPBF),
        "qm": qmf.reshape(1, N).astype(NPBF),
        "omq": (1.0 - qmf).reshape(1, N).astype(NPBF),
        "cmb": cmb,
        "cmexp": cmexp.astype(NPBF),
        "negcm": (-cmexp).astype(NPBF),
        "nkt": np.tanh(np.tile(null_key, 2)).reshape(128, 1).astype(NPBF),
        "nv": np.tile(null_value, HG).reshape(1, DG).astype(NPBF),
        "ident": np.eye(128, dtype=np.float32).astype(NPBF),
    }


def kernel(x, context, mask, context_mask, Wq, Wkv, Wo, bo, null_key, null_value):
    global LAST_RESULTS
    inputs = {
        "x": x, "context": context, "mask": mask, "context_mask": context_mask,
        "Wq": Wq, "Wkv": Wkv, "Wo": Wo, "bo": bo,
        "null_key": null_key, "null_value": null_value,
    }
    if "nc" not in _CACHE:
        _CACHE["nc"] = _build()
    nc = _CACHE["nc"]
    in_maps = [_core_inputs(inputs, core) for core in range(8)]
    res = bass_utils.run_bass_kernel_spmd(nc, in_maps, core_ids=list(range(8)))
    LAST_RESULTS = res
    bo_np = np.asarray(bo, np.float32)
    out = np.empty((B, N, DIM), np.float32)
    for b in range(B):
        out[b] = (res.results[2 * b]["out"].astype(np.float32)
                  + res.results[2 * b + 1]["out"].astype(np.float32) + bo_np)
    return out


# revision 12
# speedup vs baseline: 7.5969x; 2.4052x over previous
"""Cross-attention kernel for Trainium2, distributed over 8 NeuronCores.

Sharding: data-parallel over batch (4) x tensor-parallel over head groups (2).
Core c handles batch b = c//2, heads [4g, 4g+4) with g = c%2.

Mask-aware compaction (host-side): masked context positions contribute
exp(-50)~0 to softmax, so they are dropped before the kernel runs; masked
queries all produce the same uniform-attention row, computed host-side with
one mat-vec and scattered back. The device only processes unmasked queries
(padded to NWp, a multiple of 128) against [null | unmasked context]
(padded to W). Per-core variation (different mask counts per batch) lives
entirely in the data — zero-padded inputs plus the additive bias cmb —
so all 8 cores run one SPMD program.

All matmuls run in bf16 (4x the fp32 PE rate; fp32 accumulation in PSUM).
Per-core device pipeline:
  qT  = tanh(Wq_g^T @ x_c^T)                  [256, NWp] bf16 (d on parts)
  kT  = tanh(Wk_g^T @ ctx_c^T), null col 0    [256, W]   bf16
  v   = ctx_c @ Wv_g (+ null row, ones col)   [W, 4x65]  bf16 (j on parts)
  S^T = exp(0.125 * kT_h^T qT_h + cmb_j)      per (ic, jt, head-pair):
        scores into 2 PSUM banks, one Exp activation over both -> bf16 SBUF
  po[isub] += S_tile^T(stationary) @ v_h      [128 i, 4h x 65] PSUM
        (v as the 65-wide moving operand: 8x fewer PE cycles than moving S)
  divide by denominator column, PE-transpose O -> OT [hd, i],
  out_partial = OT^T @ Wo_g                   [NWp, 512] bf16
Host sums the two head-group partials per batch (fp32), adds bo, scatters
into the full [2048, 512] output alongside the uniform masked-query row.

The epilogue of each i-chunk (divide / transpose / out-projection) is
software-pipelined into the next chunk's score/exp/AV stream so the Exp
activations (the bottleneck engine) never stall.

PE instructions on TRN2 can carry at most ONE sync wait (walrus S3_LW /
ENGINE_NOP structs); Tile sometimes assigns more. `_split_pe_waits` runs
after scheduling and hoists extra waits onto PE nops inserted immediately
before the offending instruction.
"""

import ml_dtypes
import numpy as np

import concourse.bass as bass
import concourse.tile as tile
from concourse import bacc, bass_utils, mybir

FP = mybir.dt.float32
BF = mybir.dt.bfloat16
NPBF = np.dtype(ml_dtypes.bfloat16)
AF = mybir.ActivationFunctionType

B, N, M, DIM = 4, 2048, 2048, 512
HEADS, DH = 8, 64
G = 2          # head groups (tensor-parallel degree)
HG = 4         # heads per group
DG = HG * DH   # 256 dims per group
NEG = -50.0    # additive mask bias (exp(-50) ~ 2e-22)
SCALE = 1.0 / np.sqrt(DH)  # 0.125
VW = DH + 1    # v columns per head incl. ones column (den row)

LAST_RESULTS = None
_CACHE = {}


def _chunks(total, width=512):
    out, c0 = [], 0
    while c0 < total:
        cw = min(width, total - c0)
        out.append((c0, cw))
        c0 += cw
    return out


def _build(nwp, w):
    nc = bacc.Bacc("TRN2", debug=False, num_devices=8, enable_partition_id=False)
    d = {}

    def inp(name, shape, dt=BF):
        d[name] = nc.dram_tensor(name, shape, dt, kind="ExternalInput").ap()

    inp("xT", [DIM, nwp])
    inp("cxT", [DIM, w])
    inp("wq", [DIM, DG])
    inp("wk", [DIM, DG])
    inp("wv", [DIM, DG])
    inp("wo", [DG, DIM])
    inp("cmb", [128, w // 128], FP)  # bias: 0 attendable, NEG padding
    inp("nkt", [128, 1])             # tanh(null_key) tiled x2
    inp("nv", [1, DG])               # null_value tiled x4
    inp("ident", [128, 128])         # identity for PE transpose
    d["out"] = nc.dram_tensor("out", [nwp, DIM], BF, kind="ExternalOutput").ap()

    with tile.TileContext(nc) as tc:
        _body(tc, d, nwp, w)
    nc.compile()
    _split_pe_waits(nc)
    return nc


_SPLIT_SKIP = (
    "InstDrain", "InstUnconditionalBranch", "InstCall",
    "InstEventSemaphore", "InstRegisterMove", "InstDmaTrigger",
)


def _split_pe_waits(nc):
    """Hoist all-but-one sync waits from compute-engine instructions onto
    fresh same-engine nops placed immediately before them (TRN2 TPB
    instruction structs accept only one sync wait in walrus codegen)."""
    engines = {
        mybir.EngineType.PE: nc.tensor,
        mybir.EngineType.Activation: nc.scalar,
        mybir.EngineType.DVE: nc.vector,
        mybir.EngineType.Pool: nc.gpsimd,
        mybir.EngineType.SP: nc.sync,
    }
    total = 0
    for bb in nc.m.functions[0].blocks:
        new_insts = []
        for ins in bb.instructions:
            si = ins.sync_info
            eng = engines.get(getattr(ins, "engine", None))
            if (
                eng is not None
                and type(ins).__name__ not in _SPLIT_SKIP
                and si is not None
                and si.on_wait
                and len(si.on_wait) > 1
            ):
                waits = list(si.on_wait)
                for wt in waits[:-1]:
                    nop = eng._isa(
                        nc.isa.Opcode.NEURON_ISA_TPB_OPCODE_ENGINE_NOP,
                        {}, None, [], [], True,
                    )
                    nop.sync_info = mybir.SyncInfo(on_wait=[wt], on_update=[])
                    nc.inst_map[nop.name] = nop
                    new_insts.append(nop)
                    total += 1
                si.on_wait = waits[-1:]
            new_insts.append(ins)
        bb.instructions = new_insts
    return total


def _body(tc, d, nwp, w):
    nc = tc.nc
    jc = w // 128           # context j tiles
    ichunks = _chunks(nwp)  # 512-wide i chunks (last may be shorter)
    kchunks = _chunks(w)

    with (
        tc.tile_pool(name="consts", bufs=1) as consts,
        tc.tile_pool(name="big", bufs=1) as big,
        tc.tile_pool(name="spool", bufs=3) as spool,
        tc.tile_pool(name="small", bufs=4) as small,
        tc.tile_pool(name="sp", bufs=2, space="PSUM") as sp_ps,
        tc.tile_pool(name="po", bufs=4, space="PSUM") as po_ps,
    ):
        # ---- inputs; DMA order = first-use order ----
        wq = consts.tile([128, 4, DG], BF)
        nc.sync.dma_start(wq[:], d["wq"].rearrange("(c p) d -> p c d", p=128))

        xT = big.tile([128, 4, nwp], BF)
        xTd = d["xT"].rearrange("(c p) i -> p c i", p=128)
        cxT = big.tile([128, 4, w], BF)
        cxTd = d["cxT"].rearrange("(c p) j -> p c j", p=128)
        i0, cw = ichunks[0]
        nc.sync.dma_start(xT[:, :, i0:i0 + cw], xTd[:, :, i0:i0 + cw])

        wk = consts.tile([128, 4, DG], BF)
        nc.sync.dma_start(wk[:], d["wk"].rearrange("(c p) d -> p c d", p=128))

        for n, (c0, cw) in enumerate(kchunks):
            if n < len(ichunks) - 1:
                x0, xw = ichunks[n + 1]
                nc.sync.dma_start(xT[:, :, x0:x0 + xw], xTd[:, :, x0:x0 + xw])
            nc.sync.dma_start(cxT[:, :, c0:c0 + cw], cxTd[:, :, c0:c0 + cw])
        for n in range(len(kchunks) - 1, len(ichunks) - 1):
            x0, xw = ichunks[n + 1]
            nc.sync.dma_start(xT[:, :, x0:x0 + xw], xTd[:, :, x0:x0 + xw])

        wv = consts.tile([128, 4, DG], BF)
        nc.sync.dma_start(wv[:], d["wv"].rearrange("(c p) d -> p c d", p=128))
        wo = consts.tile([128, 2, DIM], BF)
        nc.sync.dma_start(wo[:], d["wo"].rearrange("(c p) o -> p c o", p=128))

        cmb = consts.tile([128, jc], FP)
        nc.sync.dma_start(cmb[:], d["cmb"])
        ident = consts.tile([128, 128], BF)
        nc.sync.dma_start(ident[:], d["ident"])

        qT = big.tile([128, 2, nwp], BF)
        kT = big.tile([128, 2, w], BF)
        vsb = big.tile([128, jc, HG, VW], BF)
        OsbT = big.tile([128, 2, nwp], BF)

        # ---- qT projection: qT[d, i] = tanh(sum_c Wq[c, d] x[i, c])
        # (padding queries have x = 0 -> q = 0 -> harmless, discarded on host)
        for dc in range(2):
            for n, (c0, cw) in enumerate(ichunks):
                ps = po_ps.tile([128, 512], FP, tag="po", name=f"psq{dc}{n}")
                for cc in range(4):
                    nc.tensor.matmul(
                        ps[:, 0:cw],
                        wq[:, cc, dc * 128:(dc + 1) * 128],
                        xT[:, cc, c0:c0 + cw],
                        start=(cc == 0), stop=(cc == 3),
                    )
                nc.scalar.activation(qT[:, dc, c0:c0 + cw], ps[:, 0:cw], AF.Tanh)

        # ---- kT projection (+ tanh); padding cols are tanh(0) = 0 and are
        # killed by the cmb bias anyway; null col j=0 overwritten via DMA
        for dc in range(2):
            for n, (c0, cw) in enumerate(kchunks):
                ps = po_ps.tile([128, 512], FP, tag="po", name=f"psk{dc}{n}")
                for cc in range(4):
                    nc.tensor.matmul(
                        ps[:, 0:cw],
                        wk[:, cc, dc * 128:(dc + 1) * 128],
                        cxT[:, cc, c0:c0 + cw],
                        start=(cc == 0), stop=(cc == 3),
                    )
                nc.scalar.activation(kT[:, dc, c0:c0 + cw], ps[:, 0:cw], AF.Tanh)
        for dc in range(2):
            nc.sync.dma_start(kT[:, dc, 0:1], d["nkt"])

        # ---- v projection: v[j, d]; col 64 of each head block = ones (den);
        # null row j=0 overwritten via DMA (v-proj there used cxT col 0 = 0)
        for jt in range(jc):
            ps = po_ps.tile([128, DG], FP, tag="po", name=f"psv{jt}")
            for cc in range(4):
                nc.tensor.matmul(
                    ps[:],
                    cxT[:, cc, jt * 128:(jt + 1) * 128],
                    wv[:, cc, :],
                    start=(cc == 0), stop=(cc == 3),
                )
            nc.vector.tensor_copy(
                vsb[:, jt, :, 0:DH],
                ps[:].rearrange("p (h e) -> p h e", h=HG),
            )
            nc.vector.memset(vsb[:, jt, :, DH:VW], 1.0)
        nc.sync.dma_start(vsb[0:1, 0, :, 0:DH],
                          d["nv"].rearrange("a (h e) -> a h e", h=HG))

        # ---- flash attention over i chunks, epilogue software-pipelined ----
        def emit_div(pic, po, nsub):
            Ods = []
            for isub in range(nsub):
                rden = small.tile([128, HG], FP, tag="rdn", name=f"rd{pic}{isub}")
                nc.vector.reciprocal(rden[:], po[isub][:, :, DH])
                Od = small.tile([128, HG, DH], BF, tag="od", name=f"od{pic}{isub}")
                for h in range(HG):
                    nc.vector.tensor_scalar_mul(
                        Od[:, h, :], po[isub][:, h, 0:DH], rden[:, h:h + 1])
                Ods.append(Od)
            return Ods

        def emit_transpose(pic, Ods, base, last):
            # O [i, hd] -> OT [hd, i]; both transposes share one PSUM zero
            # region via an explicit start/stop accumulation group
            for isub in range(len(Ods)):
                it0 = base + isub * 128
                pT = sp_ps.tile([128, 2, 128], BF, tag="sp",
                                name=f"pT{pic}{isub}")
                for hp in range(2):
                    nc.tensor.matmul(
                        pT[:, hp, :],
                        Ods[isub][:, 2 * hp:2 * hp + 2, :]
                        .rearrange("p h e -> p (h e)"),
                        ident[:], is_transpose=True,
                        start=(hp == 0), stop=(hp == 1),
                    )
                if last and isub % 2 == 0:
                    nc.scalar.copy(OsbT[:, :, it0:it0 + 128], pT[:])
                else:
                    nc.vector.tensor_copy(OsbT[:, :, it0:it0 + 128], pT[:])

        def emit_outproj(pic, base, nsub, last):
            for isub in range(nsub):
                it0 = base + isub * 128
                pf = sp_ps.tile([128, DIM], FP, tag="sp", name=f"pf{pic}{isub}")
                for dc in range(2):
                    nc.tensor.matmul(
                        pf[:],
                        OsbT[:, dc, it0:it0 + 128],
                        wo[:, dc, :],
                        start=(dc == 0), stop=(dc == 1),
                    )
                fo = spool.tile([128, DIM], BF, tag="fo", name=f"fo{pic}{isub}")
                if last and isub % 2 == 0:
                    nc.scalar.copy(fo[:], pf[:])
                else:
                    nc.vector.tensor_copy(fo[:], pf[:])
                nc.sync.dma_start(d["out"][it0:it0 + 128, :], fo[:])

        prev = None  # (ic index, po tiles, chunk base, nsub) pending epilogue
        for ici, (ic0, icw) in enumerate(ichunks):
            nsub = icw // 128
            pOds = emit_div(prev[0], prev[1], prev[3]) if prev else None
            po = [po_ps.tile([128, HG, VW], FP, tag="po", name=f"po{ici}{s}")
                  for s in range(nsub)]
            for jt in range(jc):
                sps, Ssb = [], []
                for hp in range(2):
                    sps.append(sp_ps.tile([128, 2, 512], FP, tag="sp",
                                          name=f"sp{ici}{jt}{hp}"))
                    for hh in range(2):
                        h = 2 * hp + hh
                        prow = DH * (h % 2)
                        nc.tensor.matmul(
                            sps[hp][:, hh, 0:icw],
                            kT[prow:prow + DH, h // 2, jt * 128:(jt + 1) * 128],
                            qT[prow:prow + DH, h // 2, ic0:ic0 + icw],
                            start=True, stop=True,
                        )
                for hp in range(2):
                    Ssb.append(spool.tile([128, 2, 512], BF, tag="s",
                                          name=f"s{ici}{jt}{hp}"))
                    nc.scalar.activation(Ssb[hp][:, :, 0:icw],
                                         sps[hp][:, :, 0:icw], AF.Exp,
                                         bias=cmb[:, jt:jt + 1],
                                         scale=float(SCALE))
                for hp in range(2):
                    for isub in range(nsub):
                        for hh in range(2):
                            h = 2 * hp + hh
                            # start=True zeroes the whole 2KB PSUM zero
                            # region: only the first matmul into each po
                            # bank may carry it
                            nc.tensor.matmul(
                                po[isub][:, h, :],
                                Ssb[hp][:, hh, isub * 128:(isub + 1) * 128],
                                vsb[:, jt, h, :],
                                start=(jt == 0 and h == 0),
                                stop=(jt == jc - 1 and h == HG - 1),
                            )
                if jt == 0 and pOds is not None:
                    emit_transpose(prev[0], pOds, prev[2], last=False)
                elif jt == 1 and pOds is not None:
                    emit_outproj(prev[0], prev[2], prev[3], last=False)
            prev = (ici, po, ic0, nsub)
        # final epilogue (nothing left to overlap with: split the copies
        # between ACT and DVE)
        Ods = emit_div(prev[0], prev[1], prev[3])
        emit_transpose(prev[0], Ods, prev[2], last=True)
        emit_outproj(prev[0], prev[2], prev[3], last=True)


def _plan(mask, context_mask):
    qidx = [np.nonzero(mask[b])[0] for b in range(B)]
    cidx = [np.nonzero(context_mask[b])[0] for b in range(B)]
    nmax = max(1, max(len(q) for q in qidx))
    mmax = max(len(c) for c in cidx) + 1  # + null at j=0
    nwp = -(-nmax // 128) * 128
    w = -(-mmax // 128) * 128
    return qidx, cidx, nwp, w


def _core_inputs(inputs, core, qidx, cidx, nwp, w):
    b, g = core // 2, core % 2
    x = np.asarray(inputs["x"], np.float32)
    context = np.asarray(inputs["context"], np.float32)
    Wq = np.asarray(inputs["Wq"], np.float32)
    Wkv = np.asarray(inputs["Wkv"], np.float32)
    Wo = np.asarray(inputs["Wo"], np.float32)
    null_key = np.asarray(inputs["null_key"], np.float32)
    null_value = np.asarray(inputs["null_value"], np.float32)

    nb, mb = len(qidx[b]), len(cidx[b])
    xc = np.zeros((nwp, DIM), np.float32)
    xc[:nb] = x[b][qidx[b]]
    cxc = np.zeros((w, DIM), np.float32)
    cxc[1:mb + 1] = context[b][cidx[b]]  # col 0 = null placeholder
    cmb = np.full(w, NEG, np.float32)
    cmb[:mb + 1] = 0.0
    gs = slice(g * DG, (g + 1) * DG)
    return {
        "xT": np.ascontiguousarray(xc.T).astype(NPBF),
        "cxT": np.ascontiguousarray(cxc.T).astype(NPBF),
        "wq": np.ascontiguousarray(Wq[:, gs]).astype(NPBF),
        "wk": np.ascontiguousarray(Wkv[:, gs]).astype(NPBF),
        "wv": np.ascontiguousarray(
            Wkv[:, DIM + g * DG: DIM + (g + 1) * DG]).astype(NPBF),
        "wo": np.ascontiguousarray(Wo[gs, :]).astype(NPBF),
        "cmb": np.ascontiguousarray(cmb.reshape(w // 128, 128).T),
        "nkt": np.tanh(np.tile(null_key, 2)).reshape(128, 1).astype(NPBF),
        "nv": np.tile(null_value, HG).reshape(1, DG).astype(NPBF),
        "ident": np.eye(128, dtype=np.float32).astype(NPBF),
    }


def kernel(x, context, mask, context_mask, Wq, Wkv, Wo, bo, null_key, null_value):
    global LAST_RESULTS
    inputs = {
        "x": x, "context": context, "mask": mask, "context_mask": context_mask,
        "Wq": Wq, "Wkv": Wkv, "Wo": Wo, "bo": bo,
        "null_key": null_key, "null_value": null_value,
    }
    mask = np.asarray(mask)
    context_mask = np.asarray(context_mask)
    qidx, cidx, nwp, w = _plan(mask, context_mask)
    key = (nwp, w)
    if key not in _CACHE:
        _CACHE[key] = _build(nwp, w)
    nc = _CACHE[key]
    in_maps = [_core_inputs(inputs, core, qidx, cidx, nwp, w)
               for core in range(8)]
    res = bass_utils.run_bass_kernel_spmd(nc, in_maps, core_ids=list(range(8)))
    LAST_RESULTS = res

    Wkv_np = np.asarray(Wkv, np.float32)
    Wo_np = np.asarray(Wo, np.float32)
    bo_np = np.asarray(bo, np.float32)
    nv_full = np.tile(np.asarray(null_value, np.float32), HEADS)
    ctx_np = np.asarray(context, np.float32)
    out = np.empty((B, N, DIM), np.float32)
    for b in range(B):
        nb = len(qidx[b])
        dev = (res.results[2 * b]["out"].astype(np.float32)
               + res.results[2 * b + 1]["out"].astype(np.float32))
        # uniform attention row for masked queries: mean over ALL 2049
        # (null + full context) values, projected through Wo
        v_sum = ctx_np[b].sum(0) @ Wkv_np[:, DIM:] + nv_full
        uniform = (v_sum / (M + 1)) @ Wo_np + bo_np
        out[b] = uniform
        if nb:
            out[b][qidx[b]] = dev[:nb] + bo_np
    return out
